# revision 17
# baseline (speedup 1.0000x reference)
# Trainium2 Bass kernel for masked causal attention
#   B=2, H=16, S=2048, D=64, bool attn_mask [B, S, S] + causal, softmax, @V.
#
# Sharding: 8 cores x 4 heads (cores 0-3 -> batch 0, cores 4-7 -> batch 1).
# Each core computes its 4 heads fully on-device; the per-batch mask is
# resident in SBUF and shared by the core's 4 heads.
#
# Per (head, k-tile kt of 128 keys):
#   S^T[k, q] = sum_d K[k,d] Q[q,d]     (PE: lhsT=K^T tile, rhs=Q^T, fp16)
#   p[k, q]   = exp(S^T/8) * mask^T     (ACT exp from PSUM -> fp16 SBUF; DVE mult)
#   outT[m,q] += sum_k vp[k,m] p[k,q]   (PE: lhsT=[V | ones] -> row 64 = denom)
# then outT[0:64]/denom via DVE reciprocal + DMA partition-broadcast + DVE mult.
# Causal structure is exploited exactly: k-tile kt only computes q >= 128*kt.

import os
import numpy as np

B, H, S, D = 2, 16, 2048, 64
NCORES = 8
HPC = 4          # heads per core
P = 128
NKT = S // P     # 16 k-tiles
CHUNK = 1024     # q-chunk size for the S^T psum tile (2 PSUM banks)
ROW_TILE = os.environ.get("ATTN_ROW_TILE", "0") == "1"
DEBUG = os.environ.get("ATTN_DEBUG", "0") == "1"

_cache = {}


def build_nc():
    import concourse.bacc as bacc
    import concourse.mybir as mybir
    import concourse.tile as tile
    from contextlib import ExitStack

    fp16 = mybir.dt.float16
    f32 = mybir.dt.float32
    Exp = mybir.ActivationFunctionType.Exp

    nc = bacc.Bacc("TRN2", target_bir_lowering=False, debug=False,
                   num_devices=NCORES)

    # Host-prepared, per-core inputs (rows 64:128 of qt/kt duplicate rows 0:64
    # so row-group tiling can be toggled without changing the host layout).
    qt_d = nc.dram_tensor("qt", [HPC, P, S], fp16, kind="ExternalInput")
    kt_d = nc.dram_tensor("kt", [HPC, P, S], fp16, kind="ExternalInput")
    vp_d = nc.dram_tensor("vp", [HPC, P, NKT, D + 1], fp16, kind="ExternalInput")
    mk_d = nc.dram_tensor("maskt", [P, NKT, S], fp16, kind="ExternalInput")
    out_d = nc.dram_tensor("outt", [HPC, D, S], f32, kind="ExternalOutput")
    if DEBUG:
        dbg_st_d = nc.dram_tensor("dbg_st", [P, CHUNK], f32, kind="ExternalOutput")
        dbg_p_d = nc.dram_tensor("dbg_p", [P, CHUNK], fp16, kind="ExternalOutput")
        dbg_num_d = nc.dram_tensor("dbg_num", [D + 1, S], f32, kind="ExternalOutput")
        dbg_rc_d = nc.dram_tensor("dbg_rc", [1, S], f32, kind="ExternalOutput")

    with tile.TileContext(nc) as tc, ExitStack() as ctx:
        mask_pool = ctx.enter_context(tc.tile_pool(name="mask", bufs=1))
        qk_pool = ctx.enter_context(tc.tile_pool(name="qk", bufs=2))
        vp_pool = ctx.enter_context(tc.tile_pool(name="vpool", bufs=2))
        p_pool = ctx.enter_context(tc.tile_pool(name="p", bufs=3))
        o_pool = ctx.enter_context(tc.tile_pool(name="osb", bufs=2))
        r_pool = ctx.enter_context(tc.tile_pool(name="recip", bufs=2))
        st_psum = ctx.enter_context(tc.tile_pool(name="st", bufs=2, space="PSUM"))
        o_psum = ctx.enter_context(tc.tile_pool(name="outp", bufs=1, space="PSUM"))
        dram_pool = ctx.enter_context(tc.tile_pool(name="dram", bufs=2, space="DRAM"))

        # Whole-batch mask^T resident for all 4 heads: [128, kt, q] fp16.
        mask_sb = mask_pool.tile([P, NKT, S], fp16, tag="mask")
        for g in range(4):
            nc.sync.dma_start(mask_sb[:, 4 * g:4 * g + 4, :],
                              mk_d[:, 4 * g:4 * g + 4, :])

        for h in range(HPC):
            nrows = P if ROW_TILE else (P // 2)
            qt = qk_pool.tile([P, S], fp16, tag="qt")
            nc.sync.dma_start(qt[0:nrows, :], qt_d[h, 0:nrows, :])
            kt = qk_pool.tile([P, S], fp16, tag="kt")
            nc.sync.dma_start(kt[0:nrows, :], kt_d[h, 0:nrows, :])
            vp = vp_pool.tile([P, NKT, D + 1], fp16, tag="vp")
            nc.sync.dma_start(vp[:], vp_d[h])

            outp = o_psum.tile([D + 1, S], f32, tag="outp")

            for j in range(NKT):
                g = (j % 2) if ROW_TILE else 0
                rlo, rhi = 64 * g, 64 * g + 64
                lhs = kt[rlo:rhi, j * P:(j + 1) * P]
                c = j * P
                while c < S:
                    e = min(S, (c // CHUNK + 1) * CHUNK)
                    w = e - c
                    stt = st_psum.tile([P, CHUNK], f32, tag="st")
                    for lo in range(0, w, 512):
                        wl = min(512, w - lo)
                        nc.tensor.matmul(stt[:, lo:lo + wl], lhsT=lhs,
                                         rhs=qt[rlo:rhi, c + lo:c + lo + wl],
                                         start=True, stop=True)
                    if DEBUG and h == 0 and j == 0 and c == 0:
                        dbg_st = p_pool.tile([P, CHUNK], f32, tag="dbgst")
                        nc.vector.tensor_copy(dbg_st[:, :w], stt[:, :w])
                        nc.sync.dma_start(dbg_st_d[:, :w], dbg_st[:, :w])
                    p = p_pool.tile([P, CHUNK], fp16, tag="p")
                    nc.scalar.activation(p[:, :w], stt[:, :w], Exp, scale=0.125)
                    nc.vector.tensor_mul(p[:, :w], p[:, :w],
                                         mask_sb[:, j, c:c + w])
                    if DEBUG and h == 0 and j == 0 and c == 0:
                        nc.sync.dma_start(dbg_p_d[:, :w], p[:, :w])
                    for b in range(c // 512, (e + 511) // 512):
                        g0, g1 = max(c, 512 * b), min(e, 512 * (b + 1))
                        nc.tensor.matmul(outp[:, g0:g1], lhsT=vp[:, j, :],
                                         rhs=p[:, g0 - c:g1 - c],
                                         start=(j == 0),
                                         stop=(j == min(4 * b + 3, NKT - 1)))
                    c = e

            if DEBUG and h == 0:
                dbg_num = o_pool.tile([D + 1, S], f32, tag="dbgnum")
                nc.vector.tensor_copy(dbg_num[:], outp[:])
                nc.sync.dma_start(dbg_num_d[:], dbg_num[:])
            # Normalize: out[0:64] / denom (denom = row 64 of outp).
            # reciprocal_approx_fast drops nonzero base partitions on HW, so
            # first copy the denom row to an SBUF tile at partition 0.
            dsb = r_pool.tile([1, S], f32, tag="dsb")
            nc.vector.tensor_copy(dsb[0:1, :], outp[D:D + 1, :])
            recip = r_pool.tile([1, S], f32, tag="recip")
            nc.vector.reciprocal_approx_fast(out=recip[0:1, :],
                                             in_=dsb[0:1, :])
            if DEBUG and h == 0:
                nc.sync.dma_start(dbg_rc_d[:], recip[0:1, :])
            rdram = dram_pool.tile([1, S], f32, tag="rdram")
            nc.sync.dma_start(rdram[:], recip[0:1, :])
            rbc = r_pool.tile([D, S], f32, tag="rbc")
            nc.sync.dma_start(rbc[:], rdram[0:1, :].to_broadcast((D, S)))
            osb = o_pool.tile([D, S], f32, tag="osb")
            nc.vector.tensor_mul(osb[:], outp[0:D, :], rbc[:])
            nc.sync.dma_start(out_d[h], osb[:])

    nc.compile()
    return nc


def prep_inputs(query, key, value, attn_mask):
    """Host-side layout prep (transposes/retiling/casts only) -> 8 in_maps."""
    query = np.asarray(query, dtype=np.float32)
    key = np.asarray(key, dtype=np.float32)
    value = np.asarray(value, dtype=np.float32)
    attn_mask = np.asarray(attn_mask).astype(bool)

    qT = np.ascontiguousarray(query.transpose(0, 1, 3, 2)).astype(np.float16)
    kT = np.ascontiguousarray(key.transpose(0, 1, 3, 2)).astype(np.float16)
    # duplicate rows for optional row-group tiling
    qTd = np.concatenate([qT, qT], axis=2)  # [B, H, 128, S]
    kTd = np.concatenate([kT, kT], axis=2)

    vp = np.concatenate(
        [value, np.ones((B, H, S, 1), np.float32)], axis=3).astype(np.float16)
    # [B, H, S, 65] -> [B, H, 128, NKT, 65] (partition-contiguous tiles)
    vp = np.ascontiguousarray(
        vp.reshape(B, H, NKT, P, D + 1).transpose(0, 1, 3, 2, 4))

    tril = np.tril(np.ones((S, S), dtype=bool))
    in_maps = []
    for b in range(B):
        m = (attn_mask[b] & tril)          # [q, k]
        mT = m.T.astype(np.float16)        # [k, q]
        maskt = np.ascontiguousarray(
            mT.reshape(NKT, P, S).transpose(1, 0, 2))  # [128, NKT, S]
        for cl in range(NCORES // B):
            h0 = cl * HPC
            in_maps.append({
                "qt": np.ascontiguousarray(qTd[b, h0:h0 + HPC]),
                "kt": np.ascontiguousarray(kTd[b, h0:h0 + HPC]),
                "vp": np.ascontiguousarray(vp[b, h0:h0 + HPC]),
                "maskt": maskt,
            })
    return in_maps


def run(query, key, value, attn_mask, trace=False, trace_cores=None):
    from concourse import bass_utils

    if "nc" not in _cache:
        _cache["nc"] = build_nc()
    nc = _cache["nc"]

    in_maps = prep_inputs(query, key, value, attn_mask)
    res = bass_utils.run_bass_kernel_spmd(
        nc, in_maps, core_ids=list(range(NCORES)),
        trace=trace, trace_cores=trace_cores)

    out = np.empty((B, H, S, D), np.float32)
    for c in range(NCORES):
        b = c // (NCORES // B)
        h0 = (c % (NCORES // B)) * HPC
        outt = res.results[c]["outt"]          # [HPC, 64, S]
        out[b, h0:h0 + HPC] = outt.transpose(0, 2, 1)
    return out, res


def kernel(query, key, value, attn_mask):
    out, _ = run(query, key, value, attn_mask)
    return out
